# revision 2
# baseline (speedup 1.0000x reference)
"""Multi-head attention (B=2, T=2048, D=1024, H=16) on 8 TRN2 NeuronCores.

Sharding: tensor-parallel over heads — 2 heads per core. Each core computes
QKV for its heads (full token range), attention, and a partial output
projection against its column shard of w_proj; the host sums the 8 partials.

Per-core device program (SPMD, identical program, per-core weight shards):
  inputs (host-prepared):
    xT  [1024, 4096]  x flattened [B*T, D] and transposed (d on partitions)
    wT  [1024, 384]   w_qkv rows for this core's heads, transposed
                      (cols: q0 q1 | k0 k1 | v0 v1, 64 each)
    wpT [128, 1024]   w_proj columns for this core's heads, transposed
  output:
    y   [4096, 1024]  partial projection output (summed across cores on host)

All matmuls run in float32r (full-rate fp32, ~1e-4 rel err). Scores are
computed transposed (scoresT[s,t]) so softmax renormalization reduces over
the PSUM partition dim via an appended ones-column in the attn@v stationary
operand; no max-subtraction is needed (|scores| < ~3 by construction).
"""

import numpy as np

import concourse.mybir as mybir
from concourse import bacc
from concourse.bass_utils import run_bass_kernel_spmd
from concourse.masks import make_identity
from concourse.tile import TileContext

F32 = mybir.dt.float32
F32R = mybir.dt.float32r

B, T, D, H = 2, 2048, 1024, 16
N_CORES = 8
HPC = H // N_CORES          # heads per core (2)
DH = D // H                 # head dim (64)
BT = B * T                  # 4096 tokens
TC = 512                    # token chunk (psum free dim)
NDT = D // 128              # 8 d-tiles
NTC = BT // TC              # 8 global token chunks
EC = 3 * HPC * DH           # 384 local qkv rows
EL = HPC * DH               # 128 local e-dims (2 heads)
VBLK = EL + HPC             # 130: v block width with 2 ones columns

_NC_CACHE = {}


def _build_nc():
    nc = bacc.Bacc("TRN2", target_bir_lowering=False, debug=False,
                   num_devices=N_CORES)
    xT = nc.dram_tensor("xT", [D, BT], F32R, kind="ExternalInput").ap()
    wT = nc.dram_tensor("wT", [D, EC], F32R, kind="ExternalInput").ap()
    wpT = nc.dram_tensor("wpT", [EL, D], F32R, kind="ExternalInput").ap()
    y = nc.dram_tensor("y", [BT, D], F32, kind="ExternalOutput").ap()

    with TileContext(nc) as tc:
        with (
            tc.tile_pool(name="const", bufs=1) as constp,
            tc.tile_pool(name="big", bufs=1) as bigp,
            tc.tile_pool(name="xin", bufs=3) as xin,
            tc.tile_pool(name="at", bufs=4) as atp,
            tc.tile_pool(name="norm", bufs=2) as normp,
            tc.tile_pool(name="on", bufs=2) as onp,
            tc.tile_pool(name="yout", bufs=4) as yp,
            tc.tile_pool(name="mm", bufs=2, space="PSUM") as mmp,
            tc.tile_pool(name="sc", bufs=4, space="PSUM") as scp,
            tc.tile_pool(name="acc", bufs=2, space="PSUM") as accp,
        ):
            ident = constp.tile([128, 128], F32)
            make_identity(nc, ident[:])
            ones_src = constp.tile([128, 1], F32)
            nc.gpsimd.memset(ones_src[:], 1.0)

            w_sb = constp.tile([128, NDT, EC], F32R)
            nc.sync.dma_start(
                out=w_sb[:], in_=wT.rearrange("(n p) e -> p n e", p=128))
            wp_sb = constp.tile([128, D], F32R)
            nc.sync.dma_start(out=wp_sb[:], in_=wpT[:])

            q_sb = bigp.tile([128, BT], F32R, tag="q")
            k_sb = bigp.tile([128, BT], F32R, tag="k")
            v_sb = bigp.tile([128, BT], F32, tag="v")
            qkv_dst = [q_sb, k_sb, v_sb]
            vbuf = bigp.tile([128, (BT // 128) * VBLK], F32R, tag="vb")
            # static ones columns of the attn@v stationary operand:
            # columns 64, 129, 194, ... (stride 65 starting at 64)
            nc.vector.tensor_copy(
                vbuf[:, DH::DH + 1],
                ones_src[:].broadcast_to([128, (BT // 128) * HPC]))

            xT_r = xT.rearrange("(n p) t -> p n t", p=128)

            # ---- Stage A: qkvT = w @ x.T, laid out [e, t] -------------------
            for ci in range(NTC):
                x_t = xin.tile([128, NDT, TC], F32R, tag="x")
                nc.sync.dma_start(
                    out=x_t[:], in_=xT_r[:, :, ci * TC:(ci + 1) * TC])
                for e in range(3):
                    ps = mmp.tile([128, TC], F32, tag="mm")
                    for d in range(NDT):
                        nc.tensor.matmul(
                            ps[:], w_sb[:, d, e * EL:(e + 1) * EL],
                            x_t[:, d, :], start=(d == 0), stop=(d == NDT - 1))
                    nc.vector.tensor_copy(
                        qkv_dst[e][:, ci * TC:(ci + 1) * TC], ps[:])

            # ---- Stage B: transpose v to [s, e] blocks with ones columns ----
            for j in range(BT // 128):
                tp = mmp.tile([128, 128], F32, tag="mm")
                nc.tensor.transpose(
                    tp[:], v_sb[:, j * 128:(j + 1) * 128], ident[:])
                dst = vbuf[:, j * VBLK:(j + 1) * VBLK].rearrange(
                    "p (g e) -> p g e", g=HPC)[:, :, 0:DH]
                src = tp[:].rearrange("p (g e) -> p g e", g=HPC)
                nc.vector.tensor_copy(dst, src)

            # ---- Stage C: attention + projection ---------------------------
            for b in range(B):
                for tci in range(T // TC):
                    g0 = b * T + tci * TC  # global token col of this chunk
                    out_ps = [accp.tile([EL // HPC + 1, TC], F32, tag="acc",
                                        name=f"acc_{b}_{tci}_{h}")
                              for h in range(HPC)]
                    for s in range(T // 128):
                        sg = b * T + s * 128
                        blk = (sg // 128) * VBLK
                        for h in range(HPC):
                            r0, r1 = h * DH, (h + 1) * DH
                            sc = scp.tile([128, TC], F32, tag="sc")
                            nc.tensor.matmul(
                                sc[:], k_sb[r0:r1, sg:sg + 128],
                                q_sb[r0:r1, g0:g0 + TC],
                                start=True, stop=True, tile_position=(r0, 0))
                            at = atp.tile([128, TC], F32R, tag="at")
                            nc.scalar.activation(
                                at[:], sc[:], mybir.ActivationFunctionType.Exp,
                                scale=float(1.0 / np.sqrt(DH)))
                            nc.tensor.matmul(
                                out_ps[h][:],
                                vbuf[:, blk + h * (DH + 1):
                                     blk + (h + 1) * (DH + 1)],
                                at[:], start=(s == 0), stop=(s == T // 128 - 1))
                    on = onp.tile([128, TC], F32R, tag="on")
                    for h in range(HPC):
                        rc = normp.tile([1, TC], F32, tag="rc")
                        nc.vector.reciprocal(rc[:], out_ps[h][DH:DH + 1, :])
                        bc = normp.tile([DH, TC], F32, tag="bc")
                        nc.gpsimd.partition_broadcast(bc[:], rc[:])
                        nc.vector.tensor_mul(
                            on[h * DH:(h + 1) * DH, :],
                            out_ps[h][0:DH, :], bc[:])
                    for tt in range(TC // 128):
                        for dc in range(D // 512):
                            yps = mmp.tile([128, 512], F32, tag="mm")
                            nc.tensor.matmul(
                                yps[:], on[:, tt * 128:(tt + 1) * 128],
                                wp_sb[:, dc * 512:(dc + 1) * 512],
                                start=True, stop=True)
                            ys = yp.tile([128, 512], F32, tag="y")
                            nc.vector.tensor_copy(ys[:], yps[:])
                            nc.sync.dma_start(
                                out=y[g0 + tt * 128:g0 + (tt + 1) * 128,
                                      dc * 512:(dc + 1) * 512],
                                in_=ys[:])

    nc.finalize()
    return nc


def _get_nc():
    if "nc" not in _NC_CACHE:
        _NC_CACHE["nc"] = _build_nc()
    return _NC_CACHE["nc"]


def kernel(x, w_qkv, w_proj):
    x = np.ascontiguousarray(x, dtype=np.float32)
    w_qkv = np.ascontiguousarray(w_qkv, dtype=np.float32)
    w_proj = np.ascontiguousarray(w_proj, dtype=np.float32)

    xT = np.ascontiguousarray(x.reshape(BT, D).T)
    in_maps = []
    for c in range(N_CORES):
        rows = slice(c * HPC * DH, (c + 1) * HPC * DH)
        w_c = np.concatenate(
            [w_qkv[0 * D:, :][rows], w_qkv[1 * D:, :][rows],
             w_qkv[2 * D:, :][rows]], axis=0)            # [384, 1024]
        wT_c = np.ascontiguousarray(w_c.T)               # [1024, 384]
        wpT_c = np.ascontiguousarray(w_proj[:, rows].T)  # [128, 1024]
        in_maps.append({"xT": xT, "wT": wT_c, "wpT": wpT_c})

    nc = _get_nc()
    res = run_bass_kernel_spmd(nc, in_maps, core_ids=list(range(N_CORES)))
    y = res.results[0]["y"].astype(np.float32)
    for c in range(1, N_CORES):
        y = y + res.results[c]["y"]
    return y.reshape(B, T, D)


# revision 6
# speedup vs baseline: 1.2429x; 1.2429x over previous
"""Multi-head attention (B=2, T=2048, D=1024, H=16) on 8 TRN2 NeuronCores.

Sharding: tensor-parallel over heads — 2 heads per core. Each core computes
QKV for its heads (full token range), attention, and a partial output
projection against its column shard of w_proj; the host sums the 8 partials.

Per-core device program (SPMD, identical program, per-core weight shards):
  inputs (host-prepared):
    xT  [1024, 4096]  x flattened [B*T, D] and transposed (d on partitions)
    wT  [1024, 384]   w_qkv rows for this core's heads, transposed
                      (cols: q0 q1 | k0 k1 | v0 v1, 64 each)
    wpT [128, 1024]   w_proj columns for this core's heads, transposed
  output:
    y   [4096, 1024]  partial projection output (summed across cores on host)

All matmuls run in float32r (full-rate fp32, ~1e-4 rel err). Scores are
computed transposed (scoresT[s,t]) so softmax renormalization reduces over
the PSUM partition dim via an appended ones-column in the attn@v stationary
operand; no max-subtraction is needed (|scores| < ~3 by construction).
"""

import numpy as np

import concourse.mybir as mybir
from concourse import bacc
from concourse.bass_utils import run_bass_kernel_spmd
from concourse.masks import make_identity
from concourse.tile import TileContext

F32 = mybir.dt.float32
F32R = mybir.dt.float32r

B, T, D, H = 2, 2048, 1024, 16
N_CORES = 8
HPC = H // N_CORES          # heads per core (2)
DH = D // H                 # head dim (64)
BT = B * T                  # 4096 tokens
TC = 512                    # token chunk (psum free dim)
NDT = D // 128              # 8 d-tiles
NTC = BT // TC              # 8 global token chunks
EC = 3 * HPC * DH           # 384 local qkv rows
EL = HPC * DH               # 128 local e-dims (2 heads)
VBLK = EL + HPC             # 130: v block width with 2 ones columns

_NC_CACHE = {}


def _build_nc():
    nc = bacc.Bacc("TRN2", target_bir_lowering=False, debug=False,
                   num_devices=N_CORES)
    xT = nc.dram_tensor("xT", [D, BT], F32R, kind="ExternalInput").ap()
    wT = nc.dram_tensor("wT", [D, EC], F32R, kind="ExternalInput").ap()
    wpT = nc.dram_tensor("wpT", [EL, D], F32R, kind="ExternalInput").ap()
    y = nc.dram_tensor("y", [BT, D], F32, kind="ExternalOutput").ap()

    with TileContext(nc) as tc:
        with (
            tc.tile_pool(name="const", bufs=1) as constp,
            tc.tile_pool(name="big", bufs=1) as bigp,
            tc.tile_pool(name="xin", bufs=3) as xin,
            tc.tile_pool(name="at", bufs=4) as atp,
            tc.tile_pool(name="norm", bufs=2) as normp,
            tc.tile_pool(name="on", bufs=2) as onp,
            tc.tile_pool(name="yout", bufs=4) as yp,
            tc.tile_pool(name="mm", bufs=2, space="PSUM") as mmp,
            tc.tile_pool(name="sc", bufs=4, space="PSUM") as scp,
            tc.tile_pool(name="acc", bufs=2, space="PSUM") as accp,
        ):
            ident = constp.tile([128, 128], F32)
            make_identity(nc, ident[:])
            ones_src = constp.tile([128, 1], F32)
            nc.gpsimd.memset(ones_src[:], 1.0)

            w_sb = constp.tile([128, NDT, EC], F32R)
            nc.sync.dma_start(
                out=w_sb[:], in_=wT.rearrange("(n p) e -> p n e", p=128))
            wp_sb = constp.tile([128, D], F32R)
            nc.sync.dma_start(out=wp_sb[:], in_=wpT[:])

            q_sb = bigp.tile([128, BT], F32R, tag="q")
            k_sb = bigp.tile([128, BT], F32R, tag="k")
            v_sb = bigp.tile([128, BT], F32, tag="v")
            qkv_dst = [q_sb, k_sb, v_sb]
            vbuf = bigp.tile([128, (BT // 128) * VBLK], F32R, tag="vb")
            # static ones columns of the attn@v stationary operand:
            # columns 64, 129, 194, ... (stride 65 starting at 64)
            nc.vector.tensor_copy(
                vbuf[:, DH::DH + 1],
                ones_src[:].broadcast_to([128, (BT // 128) * HPC]))

            xT_r = xT.rearrange("(n p) t -> p n t", p=128)

            # ---- Stage A: qkvT = w @ x.T, laid out [e, t] -------------------
            for ci in range(NTC):
                x_t = xin.tile([128, NDT, TC], F32R, tag="x")
                nc.sync.dma_start(
                    out=x_t[:], in_=xT_r[:, :, ci * TC:(ci + 1) * TC])
                for e in range(3):
                    ps = mmp.tile([128, TC], F32, tag="mm")
                    for d in range(NDT):
                        nc.tensor.matmul(
                            ps[:], w_sb[:, d, e * EL:(e + 1) * EL],
                            x_t[:, d, :], start=(d == 0), stop=(d == NDT - 1))
                    nc.vector.tensor_copy(
                        qkv_dst[e][:, ci * TC:(ci + 1) * TC], ps[:])

            # ---- Stage B: transpose v to [s, e] blocks with ones columns ----
            for j in range(BT // 128):
                tp = mmp.tile([128, 128], F32, tag="mm")
                nc.tensor.transpose(
                    tp[:], v_sb[:, j * 128:(j + 1) * 128], ident[:])
                dst = vbuf[:, j * VBLK:(j + 1) * VBLK].rearrange(
                    "p (g e) -> p g e", g=HPC)[:, :, 0:DH]
                src = tp[:].rearrange("p (g e) -> p g e", g=HPC)
                nc.vector.tensor_copy(dst, src)

            # ---- Stage C: attention + projection ---------------------------
            for b in range(B):
                for tci in range(T // TC):
                    g0 = b * T + tci * TC  # global token col of this chunk
                    out_ps = [accp.tile([EL // HPC + 1, TC], F32, tag="acc",
                                        name=f"acc_{b}_{tci}_{h}")
                              for h in range(HPC)]
                    for s in range(T // 128):
                        sg = b * T + s * 128
                        blk = (sg // 128) * VBLK
                        at = atp.tile([128, HPC * TC], F32R, tag="at")
                        for h in range(HPC):
                            r0, r1 = h * DH, (h + 1) * DH
                            sc = scp.tile([128, TC], F32, tag="sc",
                                          name=f"sc_{b}_{tci}_{s}_{h}")
                            nc.tensor.matmul(
                                sc[:],
                                k_sb[r0:r1, sg:sg + 128],
                                q_sb[r0:r1, g0:g0 + TC],
                                start=True, stop=True, tile_position=(r0, 0))
                            nc.scalar.activation(
                                at[:, h * TC:(h + 1) * TC], sc[:],
                                mybir.ActivationFunctionType.Exp,
                                scale=float(1.0 / np.sqrt(DH)))
                        for h in range(HPC):
                            nc.tensor.matmul(
                                out_ps[h][:],
                                vbuf[:, blk + h * (DH + 1):
                                     blk + (h + 1) * (DH + 1)],
                                at[:, h * TC:(h + 1) * TC],
                                start=(s == 0), stop=(s == T // 128 - 1))
                    on = onp.tile([128, TC], F32R, tag="on")
                    for h in range(HPC):
                        den = normp.tile([1, TC], F32, tag="den")
                        nc.vector.tensor_copy(den[:], out_ps[h][DH:DH + 1, :])
                        rc = normp.tile([1, TC], F32, tag="rc")
                        nc.vector.reciprocal_approx_fast(out=rc[:], in_=den[:])
                        bc = normp.tile([DH, TC], F32, tag="bc")
                        nc.gpsimd.partition_broadcast(bc[:], rc[:])
                        nc.vector.tensor_mul(
                            on[h * DH:(h + 1) * DH, :],
                            out_ps[h][0:DH, :], bc[:])
                    for tt in range(TC // 128):
                        for dc in range(D // 512):
                            yps = mmp.tile([128, 512], F32, tag="mm")
                            nc.tensor.matmul(
                                yps[:], on[:, tt * 128:(tt + 1) * 128],
                                wp_sb[:, dc * 512:(dc + 1) * 512],
                                start=True, stop=True)
                            ys = yp.tile([128, 512], F32, tag="y")
                            nc.vector.tensor_copy(ys[:], yps[:])
                            nc.sync.dma_start(
                                out=y[g0 + tt * 128:g0 + (tt + 1) * 128,
                                      dc * 512:(dc + 1) * 512],
                                in_=ys[:])

    nc.finalize()
    return nc


def _get_nc():
    if "nc" not in _NC_CACHE:
        _NC_CACHE["nc"] = _build_nc()
    return _NC_CACHE["nc"]


def kernel(x, w_qkv, w_proj):
    x = np.ascontiguousarray(x, dtype=np.float32)
    w_qkv = np.ascontiguousarray(w_qkv, dtype=np.float32)
    w_proj = np.ascontiguousarray(w_proj, dtype=np.float32)

    xT = np.ascontiguousarray(x.reshape(BT, D).T)
    in_maps = []
    for c in range(N_CORES):
        rows = slice(c * HPC * DH, (c + 1) * HPC * DH)
        w_c = np.concatenate(
            [w_qkv[0 * D:, :][rows], w_qkv[1 * D:, :][rows],
             w_qkv[2 * D:, :][rows]], axis=0)            # [384, 1024]
        wT_c = np.ascontiguousarray(w_c.T)               # [1024, 384]
        wpT_c = np.ascontiguousarray(w_proj[:, rows].T)  # [128, 1024]
        in_maps.append({"xT": xT, "wT": wT_c, "wpT": wpT_c})

    nc = _get_nc()
    res = run_bass_kernel_spmd(nc, in_maps, core_ids=list(range(N_CORES)))
    y = res.results[0]["y"].astype(np.float32)
    for c in range(1, N_CORES):
        y = y + res.results[c]["y"]
    return y.reshape(B, T, D)
